# revision 12
# baseline (speedup 1.0000x reference)
"""Additive-attention pooling (nn_Meta_Module) Trainium2 kernel — v4.

Full inputs in, full output out. Pure data-parallel over 8 NeuronCores
(batch 512 -> 64/core). Per core, a Bass/Tile kernel computes
  a    = all_memory @ U.T              (PE 128x128 phases, bf16, [k,(b,s)])
  t    = tanh(a + last @ W.T)          (bias split: ActE fused bias-tanh for
                                        4/16 batches, DVE 4D-AP bias-add +
                                        big-chunk ActE tanh for 12/16)
  sc   = V.T @ t                       (PE col-tiled phases, 2 batches/MM,
                                        4-way strip concurrency)
  P    = all_memory @ MetaW.T          (PE col-tiled phases, 2 batches/MM)
  e    = exp(sc) (ActE), esum (DVE reduce)
  out  = (sum_s e * P) / esum + Metab  (PE selector bcast + DVE mult+reduce,
                                        host divide)

Matmul phases are block-contiguous to avoid PE tiling-mode-switch drains.
"""
import numpy as np
import ml_dtypes
from contextlib import ExitStack

import concourse.bass as bass
import concourse.tile as tile
import concourse.mybir as mybir
from concourse import bacc
from concourse.bass_utils import run_bass_kernel_spmd

BF16 = mybir.dt.bfloat16
F32 = mybir.dt.float32
AF = mybir.ActivationFunctionType
ALU = mybir.AluOpType
NBF = ml_dtypes.bfloat16

B, S, H = 512, 200, 256
N_CORES = 8
B_LOC = B // N_CORES      # 64 batches/core
NW = B_LOC // 2           # 32 windows of 2 batches (400 cols)
NCOL = B_LOC * S          # 12800 columns per core
NCHUNK = 8                # x DMA chunks per half
CSZ = NCOL // NCHUNK      # 1600 cols per chunk


def _ap4(base_ap, offset_elems, dims):
    """Build a 4D AP on base_ap's tensor: dims = [(stride, count), ...3 free]."""
    p = base_ap.ap[0]
    return bass.AP(tensor=base_ap.tensor, offset=base_ap.offset + offset_elems,
                   ap=[list(p)] + [list(d) for d in dims])


def _split_cols(ap2d, outer, inner):
    """[128, outer*inner] AP -> [128, outer, inner] AP (row-major split)."""
    p = ap2d.ap[0]
    return bass.AP(tensor=ap2d.tensor, offset=ap2d.offset,
                   ap=[list(p), [inner, outer], [1, inner]])


def build_nc(b_loc=B_LOC, debug=False):
    assert b_loc == 64
    nc = bacc.Bacc("TRN2", target_bir_lowering=False, debug=debug)

    allT = [nc.dram_tensor(f"allT{h}", [128, NCOL], BF16, kind="ExternalInput")
            for h in range(2)]
    CB_d = nc.dram_tensor("CB", [128, 1152], BF16, kind="ExternalInput")
    LT_d = nc.dram_tensor("LT", [128, 2 * b_loc], F32, kind="ExternalInput")
    numer_d = nc.dram_tensor("numer", [128, 4], F32, kind="ExternalOutput")
    esum_d = nc.dram_tensor("esum", [128, 4], F32, kind="ExternalOutput")

    with tile.TileContext(nc) as tc, ExitStack() as ctx:
        big = ctx.enter_context(tc.tile_pool(name="big", bufs=1))
        misc = ctx.enter_context(tc.tile_pool(name="misc", bufs=1))
        apool = ctx.enter_context(tc.tile_pool(name="apool", bufs=2, space="PSUM"))
        vpool = ctx.enter_context(tc.tile_pool(name="vpool", bufs=2, space="PSUM"))
        ptpool = ctx.enter_context(tc.tile_pool(name="ptpool", bufs=2, space="PSUM"))

        cb = big.tile([128, 1152], BF16, tag="cb")
        nc.scalar.dma_start(cb[:], CB_d.ap())
        lt = big.tile([128, 2 * b_loc], F32, tag="lt")
        nc.scalar.dma_start(lt[:], LT_d.ap())

        ut = cb[:, 0:512]
        vsp = cb[:, 512:768]
        mwp = cb[:, 768:1024]
        sel = cb[:, 1024:1152]

        def UT(h, k):
            return ut[:, (2 * h + k) * 128:(2 * h + k + 1) * 128]

        def VSP(k, c):
            return vsp[:, (k * 4 + c) * 32:(k * 4 + c) * 32 + 32]

        def MW(h, i8):
            return mwp[:, (h * 4 + i8) * 32:(h * 4 + i8) * 32 + 32]

        # warm the exp/tanh activation table during the x DMA wait
        dummy = misc.tile([128, 1], BF16, tag="dummy")
        nc.scalar.activation(dummy[:], lt[:, 0:1], AF.Tanh)

        x = [big.tile([128, NCOL], BF16, tag=f"x{h}", name=f"x{h}")
             for h in range(2)]

        def load_chunk(c):
            eng = nc.sync if c % 2 == 0 else nc.gpsimd
            for h in range(2):
                eng.dma_start(x[h][:, c * CSZ:(c + 1) * CSZ],
                              allT[h].ap()[:, c * CSZ:(c + 1) * CSZ])

        load_chunk(0)
        load_chunk(1)

        arg = big.tile([128, 2 * NCOL], BF16, tag="arg")
        tts = [big.tile([128, NCOL], BF16, tag=f"tts{h}", name=f"tts{h}")
               for h in range(2)]

        e_full = [misc.tile([128, 2 * S], BF16, tag=f"ef{g}", name=f"ef{g}")
                  for g in range(2)]
        esum_sb = misc.tile([128, 4], F32, tag="esum")
        numer = misc.tile([128, 4], F32, tag="numer")
        pt_sb = [misc.tile([128, 2 * S], F32, tag=f"ptsb{g}", name=f"ptsb{g}")
                 for g in range(2)]
        prod = [misc.tile([128, 2 * S], F32, tag=f"prod{g}", name=f"prod{g}")
                for g in range(2)]

        vps = [None, None]
        PT = [None, None]

        def emit_V(win):
            """Scores for the 2 batches of window win -> vps[win//16]."""
            g2, wl = divmod(win, 16)
            if vps[g2] is None:
                vps[g2] = vpool.tile([128, 512], F32, tag="vps", name=f"vps{g2}")
            j, c = wl % 4, wl // 4
            first = (wl < 4)
            for k in range(2):
                nc.tensor.matmul(
                    vps[g2][32 * j:32 * j + 32, 0:2 * S], VSP(k, c),
                    tts[k][:, 400 * win:400 * (win + 1)],
                    start=(first and k == 0), stop=(k == 1),
                    tile_position=(0, 32 * j), skip_group_check=True)

        def emit_P(pg):
            """MetaW projection for the 2 batches of window pg -> PT[pg//16]."""
            g2, pl = divmod(pg, 16)
            if PT[g2] is None:
                PT[g2] = ptpool.tile([128, 512], F32, tag="pt", name=f"pt{g2}")
            j, i8 = (pl + 2) % 4, pl // 4
            first = (pl < 4)
            for h in range(2):
                nc.tensor.matmul(
                    PT[g2][32 * j:32 * j + 32, 0:2 * S], MW(h, i8),
                    x[h][:, 400 * pg:400 * (pg + 1)],
                    start=(first and h == 0), stop=(h == 1),
                    tile_position=(0, 32 * j), skip_group_check=True)

        def endgame_front(g2):
            """exp + esum + PT copy (ActE/DVE parts, no PE)."""
            nc.scalar.activation(e_full[g2][:], vps[g2][:, 0:2 * S], AF.Exp)
            nc.vector.tensor_reduce(
                esum_sb[:, 2 * g2:2 * g2 + 2], _split_cols(e_full[g2][:], 2, S),
                axis=mybir.AxisListType.X, op=ALU.add)
            nc.vector.tensor_copy(pt_sb[g2][:], PT[g2][:, 0:2 * S])

        def endgame_back(g2):
            """erep matmul (PE, 128x128 mode) + weighted reduce (DVE)."""
            erep = vpool.tile([128, 512], F32, tag="vps", name=f"erep{g2}")
            nc.tensor.matmul(erep[:, 0:2 * S], sel, e_full[g2][:],
                             start=True, stop=True)
            nc.vector.tensor_mul(prod[g2][:], pt_sb[g2][:], erep[:, 0:2 * S])
            nc.vector.tensor_reduce(
                numer[:, 2 * g2:2 * g2 + 2], _split_cols(prod[g2][:], 2, S),
                axis=mybir.AxisListType.X, op=ALU.add)

        next_chunk = [2]

        for q in range(4):                  # blocks of 8 windows
            # ---- a-phase: dense 128x128 matmuls -------------------------
            if q == 3:
                endgame_back(0)             # erep joins the 128x128 region
            for wg in range(4):             # 2-window groups
                w0 = 8 * q + 2 * wg
                w1 = w0 + 1
                if next_chunk[0] < NCHUNK and wg % 2 == 0:
                    load_chunk(next_chunk[0])
                    next_chunk[0] += 1
                at = {w: apool.tile([128, 1024], F32, tag="a", name=f"a_{w}")
                      for w in (w0, w1)}
                for k in range(2):
                    for h in range(2):
                        for w in (w0, w1):
                            nc.tensor.matmul(
                                at[w][:, 512 * k:512 * k + 400], UT(h, k),
                                x[h][:, 400 * w:400 * (w + 1)],
                                start=(h == 0), stop=(h == 1))
                # bias dispatch: groups 0..2 DVE, group 3 ActE
                for w in (w0, w1):
                    b0 = 2 * w
                    if wg != 3:
                        in0 = _ap4(at[w][:], 0,
                                   [[512, 2], [200, 2], [1, 200]])
                        out_ap = _ap4(arg[:], 400 * w,
                                      [[NCOL, 2], [200, 2], [1, 200]])
                        in1 = _ap4(lt[:], b0, [[b_loc, 2], [1, 2], [0, 200]])
                        nc.vector.tensor_tensor(out_ap, in0, in1, ALU.add)
                    else:
                        for k in range(2):
                            for i in range(2):
                                b = b0 + i
                                nc.scalar.activation(
                                    tts[k][:, S * b:S * (b + 1)],
                                    at[w][:, 512 * k + S * i:512 * k + S * (i + 1)],
                                    AF.Tanh,
                                    bias=lt[:, k * b_loc + b:k * b_loc + b + 1])
            # big biasless tanh over the block's DVE-path columns
            for k in range(2):
                lo = k * NCOL + 3200 * q
                nc.scalar.activation(tts[k][:, 3200 * q:3200 * q + 2400],
                                     arg[:, lo:lo + 2400], AF.Tanh)
            # ---- VP-phase for the previous block: col-tiled matmuls -----
            if q > 0:
                for wl in range(8):
                    win = 8 * (q - 1) + wl
                    emit_V(win)
                    emit_P(win)
                if q == 2:
                    endgame_front(0)
        # tail: VP-phase for block 3, then endgame(1)
        for wl in range(8):
            win = 24 + wl
            emit_V(win)
            emit_P(win)
        endgame_front(1)
        endgame_back(1)
        nc.sync.dma_start(numer_d.ap(), numer[:])
        nc.sync.dma_start(esum_d.ap(), esum_sb[:])
    nc.compile()
    return nc


def prep_core_inputs(all_c, last_c, U, W, V, MetaW, b_loc=B_LOC):
    x = np.ascontiguousarray(all_c.transpose(2, 0, 1)).astype(NBF)  # [H, b, S]
    m = {}
    m["allT0"] = np.ascontiguousarray(x[:128].reshape(128, b_loc * S))
    m["allT1"] = np.ascontiguousarray(x[128:].reshape(128, b_loc * S))
    l = (last_c @ W.T).astype(np.float32)
    m["LT"] = np.ascontiguousarray(
        l.T.reshape(2, 128, b_loc).transpose(1, 0, 2).reshape(128, 2 * b_loc))
    ut = U.reshape(2, 128, 2, 128).transpose(3, 2, 0, 1).reshape(128, 512)
    vsp = np.zeros((128, 256), np.float32)
    for k in range(2):
        for c in range(4):
            vsp[:, (k * 4 + c) * 32 + c] = V[128 * k:128 * (k + 1), 0]
    mwp = np.zeros((128, 2, 4, 32), np.float32)
    for h in range(2):
        for i in range(4):
            mwp[:, h, i, 4 * i:4 * i + 4] = MetaW[:, 128 * h:128 * (h + 1)].T
    mwp = mwp.reshape(128, 256)
    sel = np.zeros((128, 128), np.float32)
    for wl in range(16):
        p = 32 * (wl % 4) + wl // 4
        base = 32 * ((wl + 2) % 4) + 4 * (wl // 4)
        sel[p, base:base + 4] = 1.0
    m["CB"] = np.ascontiguousarray(
        np.concatenate([ut, vsp, mwp, sel], axis=1)).astype(NBF)
    return m


def postprocess_core(numer, esum, Metab, b_loc=B_LOC):
    out = np.empty((b_loc, 4), np.float32)
    for b in range(b_loc):
        g2, bg = divmod(b, 32)
        wl, par = divmod(bg, 2)
        prow = 32 * ((wl + 2) % 4) + 4 * (wl // 4)
        vrow = 32 * (wl % 4) + wl // 4
        out[b] = numer[prow:prow + 4, 2 * g2 + par] / esum[vrow, 2 * g2 + par]
    return out + Metab.reshape(1, 4)


_cache = {}


def _get_nc():
    if "nc" not in _cache:
        _cache["nc"] = build_nc(B_LOC)
    return _cache["nc"]


def kernel(all_memory, last_memory, U, W, V, MetaW, Metab):
    all_memory = np.asarray(all_memory, dtype=np.float32)
    last_memory = np.asarray(last_memory, dtype=np.float32)
    U = np.asarray(U, dtype=np.float32)
    W = np.asarray(W, dtype=np.float32)
    V = np.asarray(V, dtype=np.float32)
    MetaW = np.asarray(MetaW, dtype=np.float32)
    Metab = np.asarray(Metab, dtype=np.float32)
    nc = _get_nc()
    in_maps = []
    for c in range(N_CORES):
        sl = slice(c * B_LOC, (c + 1) * B_LOC)
        in_maps.append(prep_core_inputs(
            all_memory[sl], last_memory[sl], U, W, V, MetaW))
    res = run_bass_kernel_spmd(nc, in_maps, core_ids=list(range(N_CORES)))
    outs = [postprocess_core(res.results[c]["numer"], res.results[c]["esum"],
                             Metab) for c in range(N_CORES)]
    return np.concatenate(outs, axis=0).astype(np.float32)


# revision 13
# speedup vs baseline: 1.0932x; 1.0932x over previous
"""Additive-attention pooling (nn_Meta_Module) Trainium2 kernel — v4.

Full inputs in, full output out. Pure data-parallel over 8 NeuronCores
(batch 512 -> 64/core). Per core, a Bass/Tile kernel computes
  a    = all_memory @ U.T              (PE 128x128 phases, bf16, [k,(b,s)])
  t    = tanh(a + last @ W.T)          (bias split: ActE fused bias-tanh for
                                        4/16 batches, DVE 4D-AP bias-add +
                                        big-chunk ActE tanh for 12/16)
  sc   = V.T @ t                       (PE col-tiled phases, 2 batches/MM,
                                        4-way strip concurrency)
  P    = all_memory @ MetaW.T          (PE col-tiled phases, 2 batches/MM)
  e    = exp(sc) (ActE), esum (DVE reduce)
  out  = (sum_s e * P) / esum + Metab  (PE selector bcast + DVE mult+reduce,
                                        host divide)

Matmul phases are block-contiguous to avoid PE tiling-mode-switch drains.
"""
import numpy as np
import ml_dtypes
from contextlib import ExitStack

import concourse.bass as bass
import concourse.tile as tile
import concourse.mybir as mybir
from concourse import bacc
from concourse.bass_utils import run_bass_kernel_spmd

BF16 = mybir.dt.bfloat16
F32 = mybir.dt.float32
AF = mybir.ActivationFunctionType
ALU = mybir.AluOpType
NBF = ml_dtypes.bfloat16

B, S, H = 512, 200, 256
N_CORES = 8
B_LOC = B // N_CORES      # 64 batches/core
NW = B_LOC // 2           # 32 windows of 2 batches (400 cols)
NCOL = B_LOC * S          # 12800 columns per core
NCHUNK = 8                # x DMA chunks per half
CSZ = NCOL // NCHUNK      # 1600 cols per chunk


def _ap4(base_ap, offset_elems, dims):
    """Build a 4D AP on base_ap's tensor: dims = [(stride, count), ...3 free]."""
    p = base_ap.ap[0]
    return bass.AP(tensor=base_ap.tensor, offset=base_ap.offset + offset_elems,
                   ap=[list(p)] + [list(d) for d in dims])


def _split_cols(ap2d, outer, inner):
    """[128, outer*inner] AP -> [128, outer, inner] AP (row-major split)."""
    p = ap2d.ap[0]
    return bass.AP(tensor=ap2d.tensor, offset=ap2d.offset,
                   ap=[list(p), [inner, outer], [1, inner]])


def build_nc(b_loc=B_LOC, debug=False):
    assert b_loc == 64
    nc = bacc.Bacc("TRN2", target_bir_lowering=False, debug=debug)

    allT = [nc.dram_tensor(f"allT{h}", [128, NCOL], BF16, kind="ExternalInput")
            for h in range(2)]
    CB_d = nc.dram_tensor("CB", [128, 1152], BF16, kind="ExternalInput")
    LT_d = nc.dram_tensor("LT", [128, 2 * b_loc], F32, kind="ExternalInput")
    numer_d = nc.dram_tensor("numer", [128, 4], F32, kind="ExternalOutput")
    esum_d = nc.dram_tensor("esum", [128, 4], F32, kind="ExternalOutput")

    with tile.TileContext(nc) as tc, ExitStack() as ctx:
        big = ctx.enter_context(tc.tile_pool(name="big", bufs=1))
        misc = ctx.enter_context(tc.tile_pool(name="misc", bufs=1))
        apool = ctx.enter_context(tc.tile_pool(name="apool", bufs=2, space="PSUM"))
        vpool = ctx.enter_context(tc.tile_pool(name="vpool", bufs=2, space="PSUM"))
        ptpool = ctx.enter_context(tc.tile_pool(name="ptpool", bufs=2, space="PSUM"))

        cb = big.tile([128, 1152], BF16, tag="cb")
        nc.scalar.dma_start(cb[:], CB_d.ap())
        lt = big.tile([128, 2 * b_loc], F32, tag="lt")
        nc.scalar.dma_start(lt[:], LT_d.ap())

        ut = cb[:, 0:512]
        vsp = cb[:, 512:768]
        mwp = cb[:, 768:1024]
        sel = cb[:, 1024:1152]

        def UT(h, k):
            return ut[:, (2 * h + k) * 128:(2 * h + k + 1) * 128]

        def VSP(k, c):
            return vsp[:, (k * 4 + c) * 32:(k * 4 + c) * 32 + 32]

        def MW(h, i8):
            return mwp[:, (h * 4 + i8) * 32:(h * 4 + i8) * 32 + 32]

        # warm the exp/tanh activation table during the x DMA wait
        dummy = misc.tile([128, 1], BF16, tag="dummy")
        nc.scalar.activation(dummy[:], lt[:, 0:1], AF.Tanh)

        x = [big.tile([128, NCOL], BF16, tag=f"x{h}", name=f"x{h}")
             for h in range(2)]

        def load_chunk(c):
            eng = nc.sync if c % 2 == 0 else nc.gpsimd
            for h in range(2):
                eng.dma_start(x[h][:, c * CSZ:(c + 1) * CSZ],
                              allT[h].ap()[:, c * CSZ:(c + 1) * CSZ])

        load_chunk(0)
        load_chunk(1)

        arg = big.tile([128, 2 * NCOL], BF16, tag="arg")
        tts = [big.tile([128, NCOL], BF16, tag=f"tts{h}", name=f"tts{h}")
               for h in range(2)]

        e_full = [misc.tile([128, 2 * S], BF16, tag=f"ef{g}", name=f"ef{g}")
                  for g in range(2)]
        esum_sb = misc.tile([128, 4], F32, tag="esum")
        numer = misc.tile([128, 4], F32, tag="numer")
        pt_sb = [misc.tile([128, 2 * S], F32, tag=f"ptsb{g}", name=f"ptsb{g}")
                 for g in range(2)]
        prod = [misc.tile([128, 2 * S], F32, tag=f"prod{g}", name=f"prod{g}")
                for g in range(2)]

        vps = [None, None]
        PT = [None, None]

        def emit_V(win):
            """Scores for the 2 batches of window win -> vps[win//16]."""
            g2, wl = divmod(win, 16)
            if vps[g2] is None:
                vps[g2] = vpool.tile([128, 512], F32, tag="vps", name=f"vps{g2}")
            j, c = wl % 4, wl // 4
            first = (wl < 4)
            for k in range(2):
                nc.tensor.matmul(
                    vps[g2][32 * j:32 * j + 32, 0:2 * S], VSP(k, c),
                    tts[k][:, 400 * win:400 * (win + 1)],
                    start=(first and k == 0), stop=(k == 1),
                    tile_position=(0, 32 * j), skip_group_check=True)

        def emit_P(pg):
            """MetaW projection for the 2 batches of window pg -> PT[pg//16]."""
            g2, pl = divmod(pg, 16)
            if PT[g2] is None:
                PT[g2] = ptpool.tile([128, 512], F32, tag="pt", name=f"pt{g2}")
            j, i8 = (pl + 2) % 4, pl // 4
            first = (pl < 4)
            for h in range(2):
                nc.tensor.matmul(
                    PT[g2][32 * j:32 * j + 32, 0:2 * S], MW(h, i8),
                    x[h][:, 400 * pg:400 * (pg + 1)],
                    start=(first and h == 0), stop=(h == 1),
                    tile_position=(0, 32 * j), skip_group_check=True)

        def endgame_front(g2):
            """exp + esum + PT copy (ActE/DVE parts, no PE)."""
            nc.scalar.activation(e_full[g2][:], vps[g2][:, 0:2 * S], AF.Exp)
            nc.vector.tensor_reduce(
                esum_sb[:, 2 * g2:2 * g2 + 2], _split_cols(e_full[g2][:], 2, S),
                axis=mybir.AxisListType.X, op=ALU.add)
            nc.vector.tensor_copy(pt_sb[g2][:], PT[g2][:, 0:2 * S])

        def endgame_back(g2):
            """erep matmul (PE, 128x128 mode) + weighted reduce (DVE)."""
            erep = vpool.tile([128, 512], F32, tag="vps", name=f"erep{g2}")
            nc.tensor.matmul(erep[:, 0:2 * S], sel, e_full[g2][:],
                             start=True, stop=True)
            nc.vector.tensor_mul(prod[g2][:], pt_sb[g2][:], erep[:, 0:2 * S])
            nc.vector.tensor_reduce(
                numer[:, 2 * g2:2 * g2 + 2], _split_cols(prod[g2][:], 2, S),
                axis=mybir.AxisListType.X, op=ALU.add)

        next_chunk = [2]

        for q in range(4):                  # blocks of 8 windows
            # ---- a-phase: dense 128x128 matmuls -------------------------
            if q == 3:
                endgame_back(0)             # erep joins the 128x128 region
            for wg in range(4):             # 2-window groups
                w0 = 8 * q + 2 * wg
                w1 = w0 + 1
                if next_chunk[0] < NCHUNK and wg % 2 == 0:
                    load_chunk(next_chunk[0])
                    next_chunk[0] += 1
                at = {w: apool.tile([128, 1024], F32, tag="a", name=f"a_{w}")
                      for w in (w0, w1)}
                for k in range(2):
                    for h in range(2):
                        for w in (w0, w1):
                            nc.tensor.matmul(
                                at[w][:, 512 * k:512 * k + 400], UT(h, k),
                                x[h][:, 400 * w:400 * (w + 1)],
                                start=(h == 0), stop=(h == 1))
                # bias dispatch: every 4th window ActE, rest DVE
                for w in (w0, w1):
                    b0 = 2 * w
                    if w % 4 != 3:
                        in0 = _ap4(at[w][:], 0,
                                   [[512, 2], [200, 2], [1, 200]])
                        out_ap = _ap4(arg[:], 400 * w,
                                      [[NCOL, 2], [200, 2], [1, 200]])
                        in1 = _ap4(lt[:], b0, [[b_loc, 2], [1, 2], [0, 200]])
                        nc.vector.tensor_tensor(out_ap, in0, in1, ALU.add)
                    else:
                        for k in range(2):
                            for i in range(2):
                                b = b0 + i
                                nc.scalar.activation(
                                    tts[k][:, S * b:S * (b + 1)],
                                    at[w][:, 512 * k + S * i:512 * k + S * (i + 1)],
                                    AF.Tanh,
                                    bias=lt[:, k * b_loc + b:k * b_loc + b + 1])
                # big biasless tanh over this half-block's DVE-path columns
                if wg in (1, 3):
                    base = 3200 * q + 1600 * (wg // 2)
                    for k in range(2):
                        nc.scalar.activation(
                            tts[k][:, base:base + 1200],
                            arg[:, k * NCOL + base:k * NCOL + base + 1200],
                            AF.Tanh)
            # ---- VP-phase for the previous block: col-tiled matmuls -----
            if q > 0:
                for wl in range(8):
                    win = 8 * (q - 1) + wl
                    emit_V(win)
                    emit_P(win)
                if q == 2:
                    endgame_front(0)
        # tail: VP-phase for block 3, then endgame(1)
        for wl in range(8):
            win = 24 + wl
            emit_V(win)
            emit_P(win)
        endgame_front(1)
        endgame_back(1)
        nc.sync.dma_start(numer_d.ap(), numer[:])
        nc.sync.dma_start(esum_d.ap(), esum_sb[:])
    nc.compile()
    return nc


def prep_core_inputs(all_c, last_c, U, W, V, MetaW, b_loc=B_LOC):
    x = np.ascontiguousarray(all_c.transpose(2, 0, 1)).astype(NBF)  # [H, b, S]
    m = {}
    m["allT0"] = np.ascontiguousarray(x[:128].reshape(128, b_loc * S))
    m["allT1"] = np.ascontiguousarray(x[128:].reshape(128, b_loc * S))
    l = (last_c @ W.T).astype(np.float32)
    m["LT"] = np.ascontiguousarray(
        l.T.reshape(2, 128, b_loc).transpose(1, 0, 2).reshape(128, 2 * b_loc))
    ut = U.reshape(2, 128, 2, 128).transpose(3, 2, 0, 1).reshape(128, 512)
    vsp = np.zeros((128, 256), np.float32)
    for k in range(2):
        for c in range(4):
            vsp[:, (k * 4 + c) * 32 + c] = V[128 * k:128 * (k + 1), 0]
    mwp = np.zeros((128, 2, 4, 32), np.float32)
    for h in range(2):
        for i in range(4):
            mwp[:, h, i, 4 * i:4 * i + 4] = MetaW[:, 128 * h:128 * (h + 1)].T
    mwp = mwp.reshape(128, 256)
    sel = np.zeros((128, 128), np.float32)
    for wl in range(16):
        p = 32 * (wl % 4) + wl // 4
        base = 32 * ((wl + 2) % 4) + 4 * (wl // 4)
        sel[p, base:base + 4] = 1.0
    m["CB"] = np.ascontiguousarray(
        np.concatenate([ut, vsp, mwp, sel], axis=1)).astype(NBF)
    return m


def postprocess_core(numer, esum, Metab, b_loc=B_LOC):
    out = np.empty((b_loc, 4), np.float32)
    for b in range(b_loc):
        g2, bg = divmod(b, 32)
        wl, par = divmod(bg, 2)
        prow = 32 * ((wl + 2) % 4) + 4 * (wl // 4)
        vrow = 32 * (wl % 4) + wl // 4
        out[b] = numer[prow:prow + 4, 2 * g2 + par] / esum[vrow, 2 * g2 + par]
    return out + Metab.reshape(1, 4)


_cache = {}


def _get_nc():
    if "nc" not in _cache:
        _cache["nc"] = build_nc(B_LOC)
    return _cache["nc"]


def kernel(all_memory, last_memory, U, W, V, MetaW, Metab):
    all_memory = np.asarray(all_memory, dtype=np.float32)
    last_memory = np.asarray(last_memory, dtype=np.float32)
    U = np.asarray(U, dtype=np.float32)
    W = np.asarray(W, dtype=np.float32)
    V = np.asarray(V, dtype=np.float32)
    MetaW = np.asarray(MetaW, dtype=np.float32)
    Metab = np.asarray(Metab, dtype=np.float32)
    nc = _get_nc()
    in_maps = []
    for c in range(N_CORES):
        sl = slice(c * B_LOC, (c + 1) * B_LOC)
        in_maps.append(prep_core_inputs(
            all_memory[sl], last_memory[sl], U, W, V, MetaW))
    res = run_bass_kernel_spmd(nc, in_maps, core_ids=list(range(N_CORES)))
    outs = [postprocess_core(res.results[c]["numer"], res.results[c]["esum"],
                             Metab) for c in range(N_CORES)]
    return np.concatenate(outs, axis=0).astype(np.float32)
